# revision 110
# baseline (speedup 1.0000x reference)
"""Causal multi-head attention on 8 Trainium2 NeuronCores.

Problem: x[4, 2048, 1024], 16 heads of 64; q/k/v = x@W* + b*, causal
softmax attention, out = y@Wp + bp.

Sharding: core c handles batch b = c//2 and head-group hg = c%2
(8 heads = 512 feature columns of Wq/Wk/Wv, 512 rows of Wp).  Each core
computes a full [2048, 1024] partial of the output projection for its
batch; the host sums the two partials per batch and adds bp.

Per-core dataflow (bf16 matmul inputs, fp32 PSUM accumulation):
  * x arrives in DRAM as bf16; segment 0 comes in as one flat DMA and
    is transposed by the PE during its cold-start window (idle ACT/DVE
    do the PSUM copies), segments 1-3 stream in via XBAR DMA-transposes
    fully hidden behind attention.
  * qT/kT [head-pair-on-partitions, t] and v [t, heads*65] (65th column
    of each head's v block is ones so softmax denominators fall out of
    the same AV matmuls).
  * attention runs in QUARTER-passes of one head pair (hc): per
    (quarter, k-block 128, q-super-block 512) BOTH heads' score tiles
    land in one [128, 2, 512] two-bank PSUM tile and share ONE exp
    instruction (3D AP; halves the ACT per-instruction overhead, the
    second engine bottleneck).  Triangular mask on the diagonal blocks
    is one stride-0-broadcast DVE multiply for both heads.
  * AV is REORIENTED: per q-chunk of 128, y[q, 65] += sT_chunk.T @ v_ext
    (lhsT = sT chunk) streams only 65 output rows per (q128, k128) pair
    instead of 512 — half the PE time of the yT-oriented form.  The 4
    slots (2 qc x 2 heads) of a PSUM y-bank share ONE accumulation
    group (hardware zero-regions are whole banks), opened by the bank's
    first AV and closed by its last, so early q-chunks normalize while
    later k-blocks still accumulate the other bank.
  * softmax normalization is a per-partition scalar multiply (DVE) while
    copying y PSUM->SBUF bf16; y is DMA-transposed to yT for the output
    projection, whose result is staged and DMA'd to DRAM (fp32).
  * the PE instruction stream is software-pipelined by an emission
    scheduler: projection / output-projection matmuls carry deadline
    tags (first-use attention step) and are paced into the exp-limited
    attention steps by a most-binding-prefix rate, with out-projection
    work deferred into the late, exp-bound region.  Work-group PSUM
    tiles alternate across two pools so group copies never serialize.
"""
import numpy as np
from collections import deque

B, T, D = 4, 2048, 1024
NH, HD = 16, 64
NHL = 8            # heads per core
DL = NHL * HD      # 512: local qkv feature width
P = 128
QB = 512           # q super-block (columns of sT tiles)
NQ = T // QB       # 4
NKT = T // P       # 16 k blocks
KC = D // P        # 8 contraction chunks over model dim
FC = DL // P       # 4 chunks over local feature dim
DB = 512           # out-projection column block
ND = D // DB       # 2
VE = HD + 1        # 65: v block width incl ones column
LAG = 8            # exp->AV software pipeline depth (in (ik,h) steps)

_CACHE = {}


def _build():
    import concourse.bass as bass
    from concourse import bacc
    import concourse.mybir as mybir
    import concourse.tile as tile

    f32 = mybir.dt.float32
    bf16 = mybir.dt.bfloat16
    Exp = mybir.ActivationFunctionType.Exp

    nc = bacc.Bacc(None)
    x_d = nc.dram_tensor("x", [T, D], bf16, kind="ExternalInput")
    wq_d = nc.dram_tensor("wq", [D, DL], bf16, kind="ExternalInput")
    wk_d = nc.dram_tensor("wk", [D, DL], bf16, kind="ExternalInput")
    wv_d = nc.dram_tensor("wv", [D, DL], bf16, kind="ExternalInput")
    wp_d = nc.dram_tensor("wp", [DL, D], bf16, kind="ExternalInput")
    bq_d = nc.dram_tensor("bq", [DL], f32, kind="ExternalInput")
    bk_d = nc.dram_tensor("bk", [DL], f32, kind="ExternalInput")
    bv_d = nc.dram_tensor("bv", [DL], f32, kind="ExternalInput")
    mask_d = nc.dram_tensor("mask", [P, P], bf16, kind="ExternalInput")
    ident_d = nc.dram_tensor("ident", [P, P], bf16, kind="ExternalInput")
    out_d = nc.dram_tensor("out", [T, D], f32, kind="ExternalOutput")

    def bcast_ap(ap, parts):
        """Partition-broadcast view of a DRAM AP (stride-0 partition dim)."""
        return bass.AP(tensor=ap.tensor, offset=ap.offset,
                       ap=[[0, parts]] + list(ap.ap))

    with tile.TileContext(nc) as tc:
        with (
            tc.tile_pool(name="const", bufs=1) as const,
            tc.tile_pool(name="big", bufs=1) as big,
            tc.tile_pool(name="sT", bufs=20) as sT_pool,
            tc.tile_pool(name="ysb", bufs=3) as ysb_pool,
            tc.tile_pool(name="rec", bufs=4) as rec_pool,
            tc.tile_pool(name="ostage", bufs=8) as stage_pool,
            tc.tile_pool(name="ps_s", bufs=2, space="PSUM") as ps_s,
            tc.tile_pool(name="ps_y", bufs=3, space="PSUM") as ps_y,
            tc.tile_pool(name="ps_w", bufs=1, space="PSUM") as ps_w,
        ):
            # ---- persistent SBUF ----
            xT_sb = big.tile([P, KC, T], bf16)
            wq_sb = big.tile([P, KC, DL], bf16)
            wk_sb = big.tile([P, KC, DL], bf16)
            wv_sb = big.tile([P, KC, DL], bf16)
            wp_sb = big.tile([P, FC, D], bf16)
            qT_sb = big.tile([P, FC, T], bf16)
            kT_sb = big.tile([P, FC, T], bf16)
            v_sb = big.tile([P, NKT, NHL, VE], bf16)
            yT_sb = big.tile([P, FC, T], bf16)
            mask_sb = const.tile([P, P], bf16)
            ident_sb = const.tile([P, P], bf16)
            x0_sb = big.tile([P, NQ, D], bf16)
            bq_sb = const.tile([P, FC], f32)
            bk_sb = const.tile([P, FC], f32)
            bv_sb = const.tile([P, DL], f32)

            # ---- prologue DMAs (SP HWDGE queue; order = issue order,
            # sequenced to match first-use times of the PE stream) ----
            wk_r = wk_d.ap().rearrange("(c p) m -> p c m", p=P)
            wq_r = wq_d.ap().rearrange("(c p) m -> p c m", p=P)
            H = 2 * P   # weight half-width: 256 cols = 512B rows, full DMA bw
            # seg 0 of x arrives as ONE flat DMA; the PE transposes it
            # during its otherwise-idle warm-up window (the 8 XBAR
            # transposes would ring-throttle ~5us on the DMA queue)
            nc.sync.dma_start(
                x0_sb, x_d.ap()[0:QB, :].rearrange("(j p) d -> p j d", p=P))
            nc.sync.dma_start(ident_sb, ident_d.ap())
            nc.sync.dma_start(wk_sb[:, :, 0:H], wk_r[:, :, 0:H])
            nc.sync.dma_start(wq_sb[:, :, 0:H], wq_r[:, :, 0:H])
            nc.sync.dma_start(bk_sb, bk_d.ap().rearrange("(c p) -> p c", p=P))
            nc.sync.dma_start(bq_sb, bq_d.ap().rearrange("(c p) -> p c", p=P))
            nc.sync.dma_start(mask_sb, mask_d.ap())
            wv_r = wv_d.ap().rearrange("(c p) m -> p c m", p=P)
            nc.sync.dma_start(wv_sb[:, :, 0:H], wv_r[:, :, 0:H])
            nc.sync.dma_start(wk_sb[:, :, H:DL], wk_r[:, :, H:DL])
            nc.sync.dma_start(wq_sb[:, :, H:DL], wq_r[:, :, H:DL])
            nc.sync.dma_start(wv_sb[:, :, H:DL], wv_r[:, :, H:DL])
            for seg in range(1, NQ):
                for c in range(KC):
                    nc.sync.dma_start_transpose(
                        xT_sb[:, c, seg * QB:(seg + 1) * QB],
                        x_d.ap()[seg * QB:(seg + 1) * QB, c * P:(c + 1) * P])
            nc.sync.dma_start(
                wp_sb, wp_d.ap().rearrange("(c p) m -> p c m", p=P))
            nc.gpsimd.dma_start(out=bv_sb, in_=bcast_ap(bv_d.ap(), P))
            nc.vector.memset(v_sb[:, :, :, HD], 1.0)   # ones columns

            # ---- work-item machinery (each item emits ~1 PE matmul) ----
            def group_items(n_mm, emit_mm, emit_tail):
                cell = {}
                items = []
                for i in range(n_mm):
                    def it(i=i):
                        emit_mm(i, cell)
                        if i == n_mm - 1:
                            emit_tail(cell)
                    items.append(it)
                return items

            # work-group PSUM: alternate between the ps_w bank and the
            # (mostly idle between y-uses) ps_y banks so pipelined groups
            # never serialize on one bank's copy latency
            rot = [0]

            def mkps_rot():
                rot[0] ^= 1
                return (ps_w.tile([P, QB], f32, name="psw") if rot[0]
                        else ps_y.tile([P, QB], f32, name="yb"))

            def v_group(jt, hv):
                """Half of the v projection for t-block jt: feature columns
                [hv*256, (hv+1)*256) = heads 4hv..4hv+3."""
                c0, c1 = hv * H, (hv + 1) * H

                def mm(i, cell):
                    if i == 0:
                        cell["ps"] = mkps_rot()
                    nc.tensor.matmul(
                        cell["ps"][:, c0:c1],
                        lhsT=xT_sb[:, i, jt * P:(jt + 1) * P],
                        rhs=wv_sb[:, i, c0:c1],
                        start=(i == 0), stop=(i == KC - 1))

                def tail(cell):
                    nc.vector.tensor_tensor(
                        v_sb[:, jt, 4 * hv:4 * hv + 4, 0:HD],
                        cell["ps"][:, c0:c1].rearrange(
                            "p (h e) -> p h e", h=4),
                        bv_sb[:, c0:c1].rearrange("p (h e) -> p h e", h=4),
                        mybir.AluOpType.add)
                return group_items(KC, mm, tail)

            def qk_group(w_sb, b_sb, dst, m, seg, mkps=None):
                def mm(i, cell):
                    if i == 0:
                        cell["ps"] = mkps() if mkps else mkps_rot()
                    nc.tensor.matmul(
                        cell["ps"],
                        lhsT=w_sb[:, i, m * P:(m + 1) * P],
                        rhs=xT_sb[:, i, seg * QB:(seg + 1) * QB],
                        start=(i == 0), stop=(i == KC - 1))

                def tail(cell):
                    nc.vector.tensor_scalar_add(
                        dst[:, m, seg * QB:(seg + 1) * QB], cell["ps"],
                        b_sb[:, m:m + 1])
                return group_items(KC, mm, tail)

            def outproj_group(tb, nd, act_copy=False):
                def mm(c, cell):
                    if c == 0:
                        cell["ps"] = mkps_rot()
                    nc.tensor.matmul(
                        cell["ps"],
                        lhsT=yT_sb[:, c, tb * P:(tb + 1) * P],
                        rhs=wp_sb[:, c, nd * DB:(nd + 1) * DB],
                        start=(c == 0), stop=(c == FC - 1))

                def tail(cell):
                    ot = stage_pool.tile([P, DB], f32, name="ostage")
                    if act_copy:   # tail: ACT is idle after the last exp
                        nc.scalar.copy(ot, cell["ps"])
                    else:
                        nc.vector.tensor_copy(ot, cell["ps"])
                    nc.sync.dma_start(
                        out_d.ap()[tb * P:(tb + 1) * P,
                                   nd * DB:(nd + 1) * DB],
                        ot)
                return group_items(FC, mm, tail)

            def seg_groups(seg):
                gs = []
                gs += qk_group(wk_sb, bk_sb, kT_sb, 0, seg)
                gs += qk_group(wk_sb, bk_sb, kT_sb, 1, seg)
                gs += qk_group(wq_sb, bq_sb, qT_sb, 0, seg)
                gs += qk_group(wq_sb, bq_sb, qT_sb, 1, seg)
                for jt in range(seg * NQ, seg * NQ + NQ):
                    gs += v_group(jt)
                gs += qk_group(wk_sb, bk_sb, kT_sb, 2, seg)
                gs += qk_group(wk_sb, bk_sb, kT_sb, 3, seg)
                gs += qk_group(wq_sb, bq_sb, qT_sb, 2, seg)
                gs += qk_group(wq_sb, bq_sb, qT_sb, 3, seg)
                return gs

            # global step index: one step per (jq, quarter=head-pair, ik)
            base = [0]
            for j in range(NQ):
                base.append(base[-1] + 4 * (4 * j + 4))
            TOT = base[NQ]   # 160

            def sidx(jq, qt, ik=0):
                return base[jq] + qt * (4 * jq + 4) + ik

            proj_q = deque()   # items: (due_step, fn) in due order
            opt_q = deque()    # out-projection items (no deadline)
            pace = [0.0, 0.0]

            def seg_items(seg):
                """Projection work for seg, tagged with first-use steps.
                kT/qT chunk m is first read by quarter m (hc == m)."""
                its = []
                for m in range(FC):
                    its += [(sidx(seg, m), f) for f in
                            qk_group(wk_sb, bk_sb, kT_sb, m, seg)]
                    its += [(sidx(seg, m), f) for f in
                            qk_group(wq_sb, bq_sb, qT_sb, m, seg)]
                    if m < 2:
                        for jt in (seg * NQ + 2 * m, seg * NQ + 2 * m + 1):
                            for hv in range(2):
                                due = min(sidx(seg, 2 * hv, jt) + LAG + 1,
                                          sidx(seg, 2 * hv + 1) - 1)
                                its += [(due, f) for f in v_group(jt, hv)]
                its.sort(key=lambda t: t[0])
                return its

            def sprinkle(g):
                """Pace proj work by most-binding-prefix rate; spread opt
                work over all remaining steps."""
                if proj_q:
                    r = max((i + 1) / max(1, due - g)
                            for i, (due, _) in enumerate(
                                list(proj_q)[:24]))
                    pace[0] += min(r, 2.0)
                    while pace[0] >= 1.0 and proj_q:
                        proj_q.popleft()[1]()
                        pace[0] -= 1.0
                if opt_q:
                    # defer outproj into the late (exp-bound) region and
                    # hold a reserve for the ACT-bound final quarter
                    r = len(opt_q) / max(1.0, TOT + 26 - g)
                    pace[1] += r * (0.3 if g < base[2] else 1.35)
                    while pace[1] >= 1.0 and opt_q:
                        opt_q.popleft()()
                        pace[1] -= 1.0

            # ---- seg-0 transpose on the PE (idle warm-up window):
            # 4 [128,128] transposes share each bank as one accumulation
            # group (start zeroes the bank, quarters land disjoint) ----
            for cpair in range(KC // 2):
                tps = ps_s.tile([P, 2, QB], f32, name="sT_ps")
                tpb = tps.bitcast(bf16)   # [P, 2, 1024] bf16 view
                for h2 in range(2):
                    c = cpair * 2 + h2
                    for jt in range(NQ):
                        nc.tensor.matmul(
                            tpb[:, h2, jt * P:(jt + 1) * P],
                            lhsT=x0_sb[:, jt, c * P:(c + 1) * P],
                            rhs=ident_sb, is_transpose=True,
                            start=(jt == 0), stop=(jt == NQ - 1))
                    if c % 2 == 0:
                        nc.scalar.copy(xT_sb[:, c, 0:QB],
                                       tpb[:, h2, 0:QB])
                    else:
                        nc.vector.tensor_copy(xT_sb[:, c, 0:QB],
                                              tpb[:, h2, 0:QB])

            # ---- prologue: just enough of seg 0 to start attention;
            # groups run on the idle sT psum banks for deep pipelining ----
            def ps_proto():
                return ps_s.tile([P, 2, QB], f32, name="sT_ps")[:, 0, :]

            for f in qk_group(wk_sb, bk_sb, kT_sb, 0, 0, ps_proto):
                f()
            for f in qk_group(wk_sb, bk_sb, kT_sb, 1, 0, ps_proto):
                f()
            for f in qk_group(wq_sb, bq_sb, qT_sb, 0, 0, ps_proto):
                f()
            pro = []
            pro += [(sidx(0, 1), f) for f in
                    qk_group(wq_sb, bq_sb, qT_sb, 1, 0, ps_proto)]
            for m in (2, 3):
                pro += [(sidx(0, m), f) for f in
                        qk_group(wk_sb, bk_sb, kT_sb, m, 0, ps_proto)]
                pro += [(sidx(0, m), f) for f in
                        qk_group(wq_sb, bq_sb, qT_sb, m, 0, ps_proto)]
            for jt in range(NQ):
                for hv in range(2):
                    due = min(sidx(0, 2 * hv, jt) + LAG + 1,
                              sidx(0, 2 * hv + 1) - 1)
                    pro += [(due, f) for f in v_group(jt, hv)]
            pro.sort(key=lambda t: t[0])
            proj_q.extend(pro)

            # ---- attention over q super-blocks ----
            for jq in range(NQ):
                n_ik = 4 * jq + 4
                if jq + 1 < NQ:
                    proj_q.extend(seg_items(jq + 1))

                for qt in range(NQ):   # quarter = head pair (2qt, 2qt+1)
                    # y PSUM: bank b holds q-chunks 2b, 2b+1 as four
                    # 65-wide slots (2 qc x 2 heads, 65th col = denom).
                    # PSUM zero-region rule: one open accumulation group
                    # per bank — open at the bank's first AV, close at its
                    # last (precomputed from emission order), so bank 0
                    # (qc 0,1) closes early and can normalize while bank 1
                    # still accumulates.
                    ybank = [ps_y.tile([P, QB], f32, name="yb")
                             [:, 0:4 * VE].rearrange(
                                 "p (s e) -> p s e", e=VE)
                             for _ in range(2)]
                    rec = rec_pool.tile([P, 8], f32, name="rec")
                    if qt == 0:
                        ysb = ysb_pool.tile([P, NQ, DL], bf16, name="ysb")

                    avs = []   # AV emission order: (ik, parity, qc)
                    for ik_ in range(n_ik):
                        pd_ = ik_ - 4 * jq
                        for pr_ in range(2):
                            for qc_ in range(max(0, pd_), 4):
                                avs.append((ik_, pr_, qc_))
                    first_b = {}
                    last_b = {}
                    for i_, key in enumerate(avs):
                        b_ = key[2] // 2
                        first_b.setdefault(b_, i_)
                        last_b[b_] = i_
                    av_flags = {}
                    for i_, key in enumerate(avs):
                        b_ = key[2] // 2
                        av_flags[key] = (first_b[b_] == i_, last_b[b_] == i_)
                    # after how many drained AV-steps is bank b closed?
                    bank_done = {b_: next(
                        i_ for i_, key in enumerate(avs) if
                        av_flags[key][1] and key[2] // 2 == b_)
                        for b_ in (0, 1)}

                    def emit_norm(qc, jq=jq, qt=qt, ybank=ybank, rec=rec):
                        b, s0 = qc // 2, (qc % 2) * 2
                        nc.vector.reciprocal(
                            rec[:, qc * 2:(qc + 1) * 2],
                            ybank[b][:, s0:s0 + 2, HD])
                        for pr in range(2):
                            h = 2 * qt + pr
                            nc.vector.tensor_scalar_mul(
                                ysb[:, qc, h * HD:(h + 1) * HD],
                                ybank[b][:, s0 + pr, 0:HD],
                                rec[:, qc * 2 + pr:qc * 2 + pr + 1])
                        if qt == NQ - 1:
                            # qc fully normalized: transpose + queue outproj
                            nc.sync.dma_start_transpose(
                                yT_sb[:, :, jq * QB + qc * P:
                                      jq * QB + (qc + 1) * P],
                                ysb[:, qc, :])
                            for nd in range(ND):
                                opt_q.extend(outproj_group(
                                    jq * NQ + qc, nd,
                                    act_copy=(jq == NQ - 1)))

                    normed = 0
                    drained = [0]   # AV *instructions* drained
                    pending = deque()

                    def try_norm():
                        nonlocal normed
                        while (normed < NQ
                               and drained[0] > bank_done[normed // 2]):
                            emit_norm(normed)
                            normed += 1

                    for ik in range(n_ik):
                        pd = ik - 4 * jq
                        c0 = max(0, pd * P)
                        g = sidx(jq, qt, ik)
                        while proj_q and proj_q[0][0] <= g:
                            proj_q.popleft()[1]()
                        sprinkle(g)
                        while len(pending) > LAG:
                            pending.popleft()()
                        try_norm()
                        ps = ps_s.tile([P, 2, QB], f32, name="sT_ps")
                        for pr in range(2):
                            nc.tensor.matmul(
                                ps[:, pr, c0:QB],
                                lhsT=kT_sb[pr * HD:(pr + 1) * HD, qt,
                                           ik * P:(ik + 1) * P],
                                rhs=qT_sb[pr * HD:(pr + 1) * HD, qt,
                                          jq * QB + c0:(jq + 1) * QB],
                                start=True, stop=True)
                        sT = sT_pool.tile([P, 2, QB], bf16)
                        nc.scalar.activation(
                            out=sT[:, :, c0:QB], in_=ps[:, :, c0:QB],
                            func=Exp, scale=0.125)
                        if pd >= 0:
                            mb = bass.AP(
                                tensor=mask_sb.tensor, offset=mask_sb.offset,
                                ap=[mask_sb.ap[0], [0, 2], mask_sb.ap[1]])
                            nc.vector.tensor_mul(
                                sT[:, :, c0:c0 + P],
                                sT[:, :, c0:c0 + P], mb)

                        def av(ik=ik, qt=qt, sT=sT, pd=pd, jq=jq,
                               ybank=ybank, av_flags=av_flags):
                            for pr in range(2):
                                for qc in range(max(0, pd), 4):
                                    st, sp = av_flags[(ik, pr, qc)]
                                    nc.tensor.matmul(
                                        ybank[qc // 2][:, (qc % 2) * 2 + pr,
                                                       :],
                                        lhsT=sT[:, pr, qc * P:(qc + 1) * P],
                                        rhs=v_sb[:, ik, 2 * qt + pr, :],
                                        start=st, stop=sp)
                            drained[0] += (4 - max(0, pd)) * 2
                        pending.append(av)
                    # quarter ends: anything due inside it must be emitted
                    # before the remaining AVs (which may consume it)
                    g_end = sidx(jq, qt) + n_ik
                    while proj_q and proj_q[0][0] <= g_end:
                        proj_q.popleft()[1]()
                    while pending:
                        pending.popleft()()
                    try_norm()
                    while normed < NQ:
                        emit_norm(normed)
                        normed += 1

            while proj_q:
                proj_q.popleft()[1]()
            while opt_q:
                opt_q.popleft()()

    nc.finalize()
    return nc


def _in_maps(x, Wq, bq, Wk, bk, Wv, bv, Wp):
    import ml_dtypes
    bf16 = ml_dtypes.bfloat16
    mask = np.triu(np.ones((P, P), dtype=np.float32)).astype(bf16)
    maps = []
    for c in range(8):
        b, hg = divmod(c, 2)
        sl = slice(hg * DL, (hg + 1) * DL)
        maps.append({
            "x": np.ascontiguousarray(x[b]).astype(bf16),
            "wq": np.ascontiguousarray(Wq[:, sl]).astype(bf16),
            "wk": np.ascontiguousarray(Wk[:, sl]).astype(bf16),
            "wv": np.ascontiguousarray(Wv[:, sl]).astype(bf16),
            "wp": np.ascontiguousarray(Wp[sl, :]).astype(bf16),
            "bq": np.ascontiguousarray(bq[sl]),
            "bk": np.ascontiguousarray(bk[sl]),
            "bv": np.ascontiguousarray(bv[sl]),
            "mask": mask,
            "ident": np.eye(P, dtype=np.float32).astype(bf16),
        })
    return maps


def kernel(x, Wq, bq, Wk, bk, Wv, bv, Wp, bp):
    from concourse.bass_utils import run_bass_kernel_spmd

    if "nc" not in _CACHE:
        _CACHE["nc"] = _build()
    nc = _CACHE["nc"]

    x = np.asarray(x, np.float32)
    Wq, bq, Wk, bk, Wv, bv, Wp = [
        np.asarray(a, np.float32) for a in (Wq, bq, Wk, bk, Wv, bv, Wp)]
    bp = np.asarray(bp, np.float32)

    in_maps = _in_maps(x, Wq, bq, Wk, bk, Wv, bv, Wp)
    _CACHE["in_maps"] = in_maps

    res = run_bass_kernel_spmd(nc, in_maps, list(range(8))).results
    out = np.empty((B, T, D), dtype=np.float32)
    for b in range(B):
        out[b] = res[2 * b]["out"] + res[2 * b + 1]["out"] + bp
    return out
